# revision 17
# baseline (speedup 1.0000x reference)
"""Trainium2 Bass kernel for nn_ClassifierGCN (GCN conv -> z@z^T -> MLP -> sigmoid).

Contract: kernel(**inputs) takes the FULL unsharded inputs (numpy), distributes
across 8 NeuronCores internally, and returns the FULL output (numpy, f32).

Strategy (8 cores):
  - Host: build the dense edge-COUNT matrix C[src, dst] (pure index
    preprocessing; small integers -> exact in fp8, halving the adjacency
    stream), column-shard it 640 dst nodes per core (8 whole graphs/core).
    The D^-1/2 normalization ships as two tiny f32 dinv tensors.
  - Device, phase A (per core): h' = dinv_src * (x @ Wg) for ALL nodes,
    aggT_raw = h'.T @ C_slice (bf16 x fp8 matmul, f32 accum),
    zT = relu(dinv_dst * aggT_raw + bg)   [128 latent x 640 local nodes]
  - Device, phase B: per local graph g, G = z_g @ z_g^T (symmetric, [80,80]),
    flatten to DRAM; chunked AllGather -> all 64 graphs' G on every core.
  - Device, phase C: fc1/fc2 tensor-sharded along the 2n^2=12800 hidden dim
    (1600 per core): y1 = relu(Gall @ W1[:, s] + b1[s]);
    partial = y1 @ W2[s, :]  (+ b2 folded into core 0's partial, zeros elsewhere);
    chunked ReduceScatter(sum) scatters graphs back to their home cores ->
    sigmoid -> each core outputs its own 8 graphs; host concatenates.
  - bf16 for all large operands (f32 PSUM accumulation); weight streams are
    coarsened (2 K-tiles per DMA) and alternated across the two HWDGE rings,
    with deep SBUF prefetch so the streams run during phase A / the AllGather.
"""

import numpy as np
import ml_dtypes

import bass_rust
import concourse.bass as bass
import concourse.mybir as mybir
import concourse.tile as tile
from concourse.bass_utils import run_bass_kernel_spmd
from concourse.masks import make_identity
from concourse.tile_rust import add_dep_helper

# Problem shapes (hardcoded per contract).
N_NEURONS = 80
TBL = 256
LATENT = 128
N_GRAPHS = 64
N_NODES = 5120
N_CORES = 8
GPC = N_GRAPHS // N_CORES          # graphs per core = 8
DPC = N_NODES // N_CORES           # dst nodes per core = 640
N2 = N_NEURONS * N_NEURONS         # 6400
HID = 2 * N2                       # 12800
HS = HID // N_CORES                # hidden slice per core = 1600

DT = mybir.dt.bfloat16             # compute dtype for large operands
NP_DT = ml_dtypes.bfloat16
F32 = mybir.dt.float32

K_TILES_NODES = N_NODES // 128     # 40
K_TILES_N2 = N2 // 128             # 50
# fc2 contraction: 1600 = 12*128 + 64
K_TILES_HS = [(k * 128, 128) for k in range(12)] + [(1536, 64)]
N_CHUNKS_1600 = [(0, 512), (512, 512), (1024, 512), (1536, 64)]


def _fix_excess_waits(nc):
    """This container's walrus rejects >1 sem-wait on CTRL-class instructions.
    Tile's end-of-context Drain can carry several; move the excess onto NoOp
    carriers inserted just before, same engine, program order preserved."""
    n_fix = 0
    for f in nc.m.functions:
        for bb in f.blocks:
            out, changed = [], False
            for inst in bb.instructions:
                si = inst.sync_info
                waits = list(si.on_wait) if si is not None and si.on_wait else []
                if len(waits) > 1:
                    for w in waits[:-1]:
                        nop = mybir.InstNoOp(name=f"I-waitfix-{n_fix}", ins=[], outs=[])
                        n_fix += 1
                        nop.engine = inst.engine
                        nop.sync_info = bass_rust.SyncInfo(on_wait=[w], on_update=[])
                        out.append(nop)
                    si.on_wait = waits[-1:]
                    changed = True
                out.append(inst)
            if changed:
                bb.instructions = out
    return n_fix


def build_nc():
    nc = bass.Bass(num_devices=N_CORES)

    xT = nc.dram_tensor("xT", [TBL, N_NODES], DT, kind="ExternalInput")
    wg = nc.dram_tensor("wg", [TBL, LATENT], DT, kind="ExternalInput")
    bg = nc.dram_tensor("bg", [LATENT, 1], F32, kind="ExternalInput")
    ats = nc.dram_tensor("ats", [N_NODES, DPC], mybir.dt.float8e4,
                         kind="ExternalInput")
    dinv_s = nc.dram_tensor("dinv_s", [128, K_TILES_NODES], F32,
                            kind="ExternalInput")
    dinv_d = nc.dram_tensor("dinv_d", [128, DPC], F32, kind="ExternalInput")
    w1s = nc.dram_tensor("w1s", [N2, HS], DT, kind="ExternalInput")
    b1s = nc.dram_tensor("b1s", [1, HS], DT, kind="ExternalInput")
    w2s = nc.dram_tensor("w2s", [HS, N2], DT, kind="ExternalInput")
    b2s = nc.dram_tensor("b2s", [1, N2], DT, kind="ExternalInput")
    y = nc.dram_tensor("y", [GPC, N2], F32, kind="ExternalOutput")

    RG = [list(range(N_CORES))]

    with tile.TileContext(nc) as tc:
        with (
            # Weight-stream pools first so their SBUF ranges never overlap the
            # phase-A pools -> prefetch can run from t=0.
            tc.tile_pool(name="wp", bufs=24) as wpool,
            tc.tile_pool(name="const", bufs=1) as constp,
            tc.tile_pool(name="persist", bufs=1) as persist,
            tc.tile_pool(name="dram", bufs=1, space="DRAM") as dramp,
        ):
            # Constants.
            ident = constp.tile([64, 64], DT)
            make_identity(nc, ident[:])
            ones1 = constp.tile([1, 64], DT)
            nc.gpsimd.memset(ones1[:], 1.0)
            bg_sb = constp.tile([LATENT, 1], F32)
            dinv_s_sb = constp.tile([128, K_TILES_NODES], F32)
            dinv_d_sb = constp.tile([128, DPC], F32)
            b1_sb = constp.tile([1, HS], DT)
            b2_sb = constp.tile([1, N2], DT)

            # Persistent SBUF tensors.
            zT = persist.tile([128, DPC], DT)                        # [latent, local node]
            gT_big = persist.tile([128, K_TILES_N2 * 64], DT)        # vec(G) K-tiles x 64 graphs
            y1T_big = persist.tile([128, len(K_TILES_HS) * 64], DT)  # y1^T K-tiles x 64 graphs
            y1_sb = persist.tile([N_GRAPHS, HS], DT)

            # DRAM bounce buffers for the (chunked) collectives.
            AG_CHUNKS = 2
            AGW = N2 // AG_CHUNKS      # 3200
            AGR = AGW // N_NEURONS     # 40 G-rows per chunk
            g_loc = [dramp.tile([GPC, AGW], DT, name=f"g_loc{c}")
                     for c in range(AG_CHUNKS)]
            g_all = [dramp.tile([N_GRAPHS, AGW], DT, addr_space="Shared",
                                name=f"g_all{c}")
                     for c in range(AG_CHUNKS)]
            RS_W = [3200, 3200]         # RS after fc2 chunks 1 and 3
            RS_OF = [0, 3200]
            y_loc = [dramp.tile([N_GRAPHS, w], DT, name=f"y_loc{c}")
                     for c, w in enumerate(RS_W)]
            y_red = [dramp.tile([GPC, w], DT, name=f"y_red{c}")
                     for c, w in enumerate(RS_W)]

            # ---- Phase A (fused): h = x @ Wg, aggT = h.T @ ATs, streamed
            #      through small recycled pools (each x / h tile used once) ----
            with (
                tc.tile_pool(name="xa", bufs=3) as xap,
                tc.tile_pool(name="hp", bufs=8) as hpool,
                tc.tile_pool(name="atp", bufs=2) as atp,
                tc.tile_pool(name="hps", bufs=2, space="PSUM") as hps,
                tc.tile_pool(name="aggps", bufs=1, space="PSUM") as aggps,
            ):
                wg_sb = xap.tile([128, 2, LATENT], DT, tag="wg", bufs=1)
                nc.sync.dma_start(
                    wg_sb[:], wg[:, :].rearrange("(a b) c -> b a c", a=2))
                agg = aggps.tile([128, DPC], F32)
                xt_c = None
                at5 = None
                XT_CHUNKS = [(0, 2), (2, 8), (10, 10), (20, 10), (30, 10)]
                xt_starts = {c0: n for c0, n in XT_CHUNKS}
                for k in range(K_TILES_NODES):
                    if k in xt_starts:
                        n = xt_starts[k]
                        cs, ce = k * 128, (k + n) * 128
                        xt_c = xap.tile([128, 2, 1280], DT, tag="xt")
                        k0 = k
                        xt_dma = nc.sync.dma_start(
                            xt_c[:, :, 0:n * 128],
                            xT[:, cs:ce].rearrange("(a b) c -> b a c", a=2))
                        if k == 0:
                            nc._first_xt_dma = xt_dma.ins
                            # needed first at the k=0 h-scale on DVE; traced
                            # here so it rides behind the critical loads
                            nc.sync.dma_start(dinv_s_sb[:], dinv_s[:, :])
                    mm = (k - k0) * 128
                    ph = hps.tile([128, 128], F32)
                    nc.tensor.matmul(ph[:], lhsT=xt_c[:, 0, mm:mm + 128],
                                     rhs=wg_sb[:, 0, :], start=True, stop=False)
                    nc.tensor.matmul(ph[:], lhsT=xt_c[:, 1, mm:mm + 128],
                                     rhs=wg_sb[:, 1, :], start=False, stop=True)
                    h_t = hpool.tile([128, 128], DT)
                    nc.vector.tensor_scalar_mul(h_t[:], ph[:],
                                                dinv_s_sb[:, k:k + 1])
                    if k % 5 == 0:
                        kb = k // 5
                        at5 = atp.tile([128, 5, DPC], mybir.dt.float8e4)
                        src = ats[kb * 640:(kb + 1) * 640, :].rearrange(
                            "(a b) c -> b a c", a=5)
                        nc.sync.dma_start(at5[:], src)
                    st = (k == 0)
                    sp = (k == K_TILES_NODES - 1)
                    nc.tensor.matmul(agg[:, 0:512], lhsT=h_t[:],
                                     rhs=at5[:, k % 5, 0:512], start=st, stop=sp)
                    nc.tensor.matmul(agg[:, 512:640], lhsT=h_t[:],
                                     rhs=at5[:, k % 5, 512:640], start=st, stop=sp)
                nc.sync.dma_start(bg_sb[:], bg[:, :])
                nc.sync.dma_start(dinv_d_sb[:], dinv_d[:, :])
                nc.sync.dma_start(b1_sb[:], b1s[:, :])
                nc.sync.dma_start(b2_sb[:], b2s[:, :])
                aggs = xap.tile([128, DPC], F32, tag="aggs", bufs=1)
                nc.vector.tensor_tensor(aggs[:], agg[:], dinv_d_sb[:],
                                        op=mybir.AluOpType.mult)
                nc.scalar.activation(zT[:], aggs[:],
                                     mybir.ActivationFunctionType.Relu,
                                     bias=bg_sb[:, 0:1])

            # ---- Phase B: per-graph G = z z^T, flatten, chunked AllGather ----
            with (
                tc.tile_pool(name="gps", bufs=2, space="PSUM") as gps,
                tc.tile_pool(name="gsb", bufs=1) as gsbp,
            ):
                gsb_all = gsbp.tile([N_NEURONS, GPC * N_NEURONS], DT)
                for g in range(GPC):
                    gp = gps.tile([N_NEURONS, N_NEURONS], F32)
                    zg = zT[:, g * N_NEURONS:(g + 1) * N_NEURONS]
                    nc.tensor.matmul(gp[:], lhsT=zg, rhs=zg, start=True, stop=True)
                    nc.vector.tensor_copy(
                        gsb_all[:, g * N_NEURONS:(g + 1) * N_NEURONS], gp[:])
                for c in range(AG_CHUNKS):
                    # g_loc[c][g, r*80+col] = G_g[c*AGR + r, col]; two halves on
                    # the two rings so neither waits behind a full weight pair
                    half = AGR // 2
                    for hh, eng in ((0, nc.sync), (1, nc.scalar)):
                        r0 = c * AGR + hh * half
                        dst = g_loc[c][:, hh * half * 80:(hh + 1) * half * 80]
                        eng.dma_start(
                            dst.rearrange("g (r c) -> r g c", r=half),
                            gsb_all[r0:r0 + half, :].rearrange(
                                "r (g c) -> r g c", g=GPC))
                    nc.gpsimd.collective_compute(
                        "AllGather", mybir.AluOpType.bypass, replica_groups=RG,
                        ins=[g_loc[c].opt()], outs=[g_all[c].opt()],
                    )

            # ---- Phase C0: transpose Gall into [128 x 64] K-tiles ----
            with (
                tc.tile_pool(name="gallp", bufs=4) as gallp,
                tc.tile_pool(name="tps", bufs=4, space="PSUM") as tps,
            ):
                for blk in range(10):           # 10 loads of [64, 640]
                    c, b = divmod(blk, 5)
                    ga = gallp.tile([N_GRAPHS, 640], DT)
                    nc.sync.dma_start(ga[:], g_all[c][:, b * 640:(b + 1) * 640])
                    for j in range(5):
                        t = blk * 5 + j
                        tp = tps.tile([128, N_GRAPHS], DT)
                        nc.tensor.transpose(tp[:], ga[:, j * 128:(j + 1) * 128],
                                            ident[:])
                        nc.vector.tensor_copy(gT_big[:, t * 64:(t + 1) * 64], tp[:])

                # ---- Phase C1: y1 = relu(Gall @ W1s + b1s) ----
                with tc.tile_pool(name="y1ps", bufs=1, space="PSUM") as y1psp:
                    y1ps = y1psp.tile([N_GRAPHS, HS], F32)
                    for (n0, nw) in N_CHUNKS_1600:
                        nc.tensor.matmul(y1ps[:, n0:n0 + nw], lhsT=ones1[:],
                                         rhs=b1_sb[:, n0:n0 + nw],
                                         start=True, stop=False)
                    for kp in range(K_TILES_N2 // 2):
                        w1t = wpool.tile([128, 2, HS], DT, tag="w")
                        eng = nc.scalar if kp % 2 == 0 else nc.sync
                        w1_dma = eng.dma_start(
                            w1t[:],
                            w1s[kp * 256:(kp + 1) * 256, :].rearrange(
                                "(a b) c -> b a c", a=2))
                        if kp <= 1:
                            # keep the first weight-pair transfers from landing
                            # ahead of the phase-A-critical first xT chunk
                            add_dep_helper(w1_dma.ins, nc._first_xt_dma,
                                           sync=True,
                                           reason="w-stream after first xT")
                        for kk in range(2):
                            k = kp * 2 + kk
                            lhs = gT_big[:, k * 64:(k + 1) * 64]
                            for (n0, nw) in N_CHUNKS_1600:
                                nc.tensor.matmul(y1ps[:, n0:n0 + nw], lhsT=lhs,
                                                 rhs=w1t[:, kk, n0:n0 + nw],
                                                 start=False,
                                                 stop=(k == K_TILES_N2 - 1))
                    for (n0, nw) in N_CHUNKS_1600:
                        nc.scalar.activation(y1_sb[:, n0:n0 + nw],
                                             y1ps[:, n0:n0 + nw],
                                             mybir.ActivationFunctionType.Relu)

                # ---- Phase C2: transpose y1 into K-tiles ----
                for t, (k0, kw) in enumerate(K_TILES_HS):
                    tp = tps.tile([128, N_GRAPHS], DT)
                    nc.tensor.transpose(tp[0:kw, :], y1_sb[:, k0:k0 + kw], ident[:])
                    nc.vector.tensor_copy(y1T_big[0:kw, t * 64:(t + 1) * 64], tp[0:kw, :])

            # ---- Phase C3: fc2 partial = y1 @ W2s (+ b2 on core 0), chunked;
            #      ReduceScatter after chunks 1 and 3, sigmoid + store ----
            with (
                tc.tile_pool(name="p2ps", bufs=2, space="PSUM") as p2psp,
                tc.tile_pool(name="y2sb", bufs=2) as y2sbp,
                tc.tile_pool(name="sig", bufs=4) as sigp,
            ):
                for c in range(4):
                    c0 = c * 1600
                    p2 = p2psp.tile([N_GRAPHS, 1600], F32)
                    for (n0, nw) in N_CHUNKS_1600:
                        nc.tensor.matmul(p2[:, n0:n0 + nw], lhsT=ones1[:],
                                         rhs=b2_sb[:, c0 + n0:c0 + n0 + nw],
                                         start=True, stop=False)
                    for tp2 in range(7):
                        eng = nc.scalar if (c * 7 + tp2) % 2 == 0 else nc.sync
                        if tp2 < 6:
                            w2t = wpool.tile([128, 2, 1600], DT, tag="w")
                            eng.dma_start(
                                w2t[:],
                                w2s[tp2 * 256:(tp2 + 1) * 256,
                                    c0:c0 + 1600].rearrange("(a b) c -> b a c", a=2))
                            pieces = [(tp2 * 2, 0, 128), (tp2 * 2 + 1, 1, 128)]
                        else:
                            w2t = wpool.tile([128, 2, 1600], DT, tag="w")
                            eng.dma_start(w2t[0:64, 0, :],
                                          w2s[1536:1600, c0:c0 + 1600])
                            pieces = [(12, 0, 64)]
                        for (t, kk, kw) in pieces:
                            lhs = y1T_big[0:kw, t * 64:(t + 1) * 64]
                            for (n0, nw) in N_CHUNKS_1600:
                                nc.tensor.matmul(p2[:, n0:n0 + nw], lhsT=lhs,
                                                 rhs=w2t[0:kw, kk, n0:n0 + nw],
                                                 start=False, stop=(t == 12))
                    y2sb = y2sbp.tile([N_GRAPHS, 1600], DT)
                    for (n0, nw) in N_CHUNKS_1600:
                        nc.vector.tensor_copy(y2sb[:, n0:n0 + nw],
                                              p2[:, n0:n0 + nw])
                    r = c // 2
                    rc0 = c0 - RS_OF[r]
                    nc.sync.dma_start(y_loc[r][:, rc0:rc0 + 1600], y2sb[:])
                    if c % 2 == 1:
                        w = RS_W[r]
                        nc.gpsimd.collective_compute(
                            "ReduceScatter", mybir.AluOpType.add, replica_groups=RG,
                            ins=[y_loc[r].opt()], outs=[y_red[r].opt()],
                        )
                        # sigmoid over all 128 partitions: [8, w] -> [128, w/16]
                        w16 = w // 16
                        ys = sigp.tile([128, 200], DT, tag="ys")
                        nc.sync.dma_start(
                            ys[:, 0:w16],
                            y_red[r][:, :].rearrange("g (j t) -> g j t", j=16))
                        yo = sigp.tile([128, 200], F32, tag="yo")
                        nc.scalar.activation(yo[:, 0:w16], ys[:, 0:w16],
                                             mybir.ActivationFunctionType.Sigmoid)
                        nc.sync.dma_start(
                            y[:, RS_OF[r]:RS_OF[r] + w].rearrange(
                                "g (j t) -> g j t", j=16), yo[:, 0:w16])

    _fix_excess_waits(nc)
    return nc


_NC_CACHE = None


def _get_nc():
    global _NC_CACHE
    if _NC_CACHE is None:
        _NC_CACHE = build_nc()
    return _NC_CACHE


def prep_in_maps(x, edge_index, Wg, bg, W1, b1, W2, b2):
    x = np.asarray(x, np.float32)
    edge_index = np.asarray(edge_index)
    Wg = np.asarray(Wg, np.float32)
    bg = np.asarray(bg, np.float32)
    W1 = np.asarray(W1, np.float32)
    b1 = np.asarray(b1, np.float32)
    W2 = np.asarray(W2, np.float32)
    b2 = np.asarray(b2, np.float32)

    src = edge_index[0].astype(np.int64)
    dst = edge_index[1].astype(np.int64)

    # Degree / normalization (index preprocessing, matches reference formula).
    deg = np.bincount(dst, minlength=N_NODES).astype(np.float32)
    dinv = np.where(deg > 0, 1.0 / np.sqrt(np.maximum(deg, 1.0)), 0.0).astype(np.float32)

    # Dense edge-count matrix, laid out [src, dst]. Counts are small integers
    # -> exactly representable in fp8; the dinv normalization is applied on
    # device in f32, so this halves the adjacency stream with no extra error.
    # (bincount over flat indices is ~10x faster than np.add.at here)
    counts = np.bincount(src * N_NODES + dst, minlength=N_NODES * N_NODES)
    at = counts.astype(ml_dtypes.float8_e4m3).reshape(N_NODES, N_NODES)
    dinv_s_np = np.ascontiguousarray(
        dinv.reshape(K_TILES_NODES, 128).T)          # [128, 40]

    xT = np.ascontiguousarray(x.T).astype(NP_DT)
    wg_np = Wg.astype(NP_DT)
    bg_np = np.ascontiguousarray(bg.reshape(LATENT, 1))

    in_maps = []
    for c in range(N_CORES):
        s0 = c * HS
        b2c = b2 if c == 0 else np.zeros_like(b2)
        in_maps.append({
            "xT": xT,
            "wg": wg_np,
            "bg": bg_np,
            "ats": np.ascontiguousarray(at[:, c * DPC:(c + 1) * DPC]),
            "dinv_s": dinv_s_np,
            "dinv_d": np.ascontiguousarray(np.broadcast_to(
                dinv[c * DPC:(c + 1) * DPC], (128, DPC))),
            "w1s": np.ascontiguousarray(W1[:, s0:s0 + HS]).astype(NP_DT),
            "b1s": np.ascontiguousarray(b1[s0:s0 + HS].reshape(1, HS)).astype(NP_DT),
            "w2s": np.ascontiguousarray(W2[s0:s0 + HS, :]).astype(NP_DT),
            "b2s": np.ascontiguousarray(b2c.reshape(1, N2)).astype(NP_DT),
        })
    return in_maps


def kernel(x, edge_index, Wg, bg, W1, b1, W2, b2):
    in_maps = prep_in_maps(x, edge_index, Wg, bg, W1, b1, W2, b2)
    nc = _get_nc()
    res = run_bass_kernel_spmd(nc, in_maps, core_ids=list(range(N_CORES)))
    out = np.concatenate([res.results[c]["y"] for c in range(N_CORES)], axis=0)
    return out.reshape(-1).astype(np.float32)



# revision 18
# speedup vs baseline: 1.7956x; 1.7956x over previous
"""Trainium2 Bass kernel for nn_ClassifierGCN (GCN conv -> z@z^T -> MLP -> sigmoid).

Contract: kernel(**inputs) takes the FULL unsharded inputs (numpy), distributes
across 8 NeuronCores internally, and returns the FULL output (numpy, f32).

Strategy (8 cores), v2 — fp8 everywhere it's safe + symmetric-G triangle:
  - Host: dense edge matrix ats[src, dst] = count * dinv_src * 8 (fp8e4m3),
    column-sharded 640 dst nodes (8 whole graphs) per core.
  - Phase A (device): h = x @ Wg (fp8 DoubleRow), h' = h*4 -> fp8;
    aggT += h'.T @ ats  (fp8 DoubleRow over node-tile pairs);
    zT = relu(aggT * dinv_dst/8) * 4 + bg  [128 latent x 640 local nodes] bf16.
  - Phase B: G = z z^T is symmetric; compute only block-triangle (5x5 blocks
    of 16, pairs a<=b -> 3840 of 6400 entries). W1 rows for (i,j)/(j,i) are
    pre-summed on host, so fc1 contraction shrinks 6400 -> 3840 EXACTLY.
    Pack vec(G-tri) (x16 via z scale) to fp8 DRAM, single AllGather.
  - Phase C: fc1/fc2 tensor-sharded along the 12800 hidden dim (1600/core),
    both computed TRANSPOSED (out [dim, 64 graphs]) so matmul cost (ap_size =
    out free = 64) is minimal; fp8 DoubleRow over k-tile pairs. b2 seeded via
    ones-matmul (core 0). Partials [128, 3200] bf16 -> single ReduceScatter
    (16 partition-rows per core) -> sigmoid(x/1024) -> y [128, 400] f32.
  - Host reassembles the n2-index mapping from the scattered partition rows.
"""

import numpy as np
import ml_dtypes

import bass_rust
import concourse.bass as bass
import concourse.mybir as mybir
import concourse.tile as tile
from concourse.bass_utils import run_bass_kernel_spmd
from concourse.masks import make_identity
from concourse.tile_rust import add_dep_helper

# Problem shapes (hardcoded per contract).
N_NEURONS = 80
TBL = 256
LATENT = 128
N_GRAPHS = 64
N_NODES = 5120
N_CORES = 8
GPC = N_GRAPHS // N_CORES          # graphs per core = 8
DPC = N_NODES // N_CORES           # dst nodes per core = 640
N2 = N_NEURONS * N_NEURONS         # 6400
HID = 2 * N2                       # 12800
HS = HID // N_CORES                # hidden slice per core = 1600

# symmetric-G triangle blocking: 5 blocks of 16 neurons, pairs a<=b
BLK = 16
NB = N_NEURONS // BLK              # 5
C0 = [0, 80, 144, 192, 224]        # col offset of a-row's blocks in gtri
CW = 240                           # gtri cols per partition row
NTRI = 16 * CW                     # 3840 packed G entries
K1 = NTRI // 128                   # 30 fc1 k-tiles -> 15 DoubleRow pairs
M1 = 13                            # fc1 m-tiles (12x128 + 64)
M2 = N2 // 128                     # 50 fc2 m-tiles
KP2 = 6                            # fc2 full DoubleRow pairs (6*256=1536)

F8 = mybir.dt.float8e4
BF16 = mybir.dt.bfloat16
F32 = mybir.dt.float32
NP_F8 = ml_dtypes.float8_e4m3fn
NP_BF16 = ml_dtypes.bfloat16
DR = mybir.MatmulPerfMode.DoubleRow

# fixed power-of-2 quantization scales (host folds them into the inputs)
S_H = 4.0        # h stored *4
S_A = 8.0        # ats stored = count*dinv_src*8
S_Z = 4.0        # z stored *4  -> G psum is *16
S_W1 = 128.0     # W1' stored *128 -> fc1 psum = y1pre * 16*128 = *2048
S_Y1 = 8.0       # y1 stored *8  (act scale 8/2048 = 1/256)
S_W2 = 128.0     # W2 stored *128 -> fc2 psum = z2 * 8*128 = *1024


def _fix_excess_waits(nc):
    """This container's walrus rejects >1 sem-wait on CTRL-class instructions.
    Tile's end-of-context Drain can carry several; move the excess onto NoOp
    carriers inserted just before, same engine, program order preserved."""
    n_fix = 0
    for f in nc.m.functions:
        for bb in f.blocks:
            out, changed = [], False
            for inst in bb.instructions:
                si = inst.sync_info
                waits = list(si.on_wait) if si is not None and si.on_wait else []
                if len(waits) > 1:
                    for w in waits[:-1]:
                        nop = mybir.InstNoOp(name=f"I-waitfix-{n_fix}", ins=[], outs=[])
                        n_fix += 1
                        nop.engine = inst.engine
                        nop.sync_info = bass_rust.SyncInfo(on_wait=[w], on_update=[])
                        out.append(nop)
                    si.on_wait = waits[-1:]
                    changed = True
                out.append(inst)
            if changed:
                bb.instructions = out
    return n_fix


def build_nc():
    nc = bass.Bass(num_devices=N_CORES)

    xT = nc.dram_tensor("xT", [TBL, N_NODES], F8, kind="ExternalInput")
    wg = nc.dram_tensor("wg", [TBL, LATENT], F8, kind="ExternalInput")
    ats = nc.dram_tensor("ats", [N_NODES, DPC], F8, kind="ExternalInput")
    dinv_d = nc.dram_tensor("dinv_d", [128, DPC], F32, kind="ExternalInput")
    bg4 = nc.dram_tensor("bg4", [LATENT, 1], F32, kind="ExternalInput")
    w1s = nc.dram_tensor("w1s", [NTRI, HS], F8, kind="ExternalInput")
    b1s = nc.dram_tensor("b1s", [128, M1], F32, kind="ExternalInput")
    w2s = nc.dram_tensor("w2s", [HS, N2], F8, kind="ExternalInput")
    b2s = nc.dram_tensor("b2s", [1, N2], BF16, kind="ExternalInput")
    y = nc.dram_tensor("y", [128, 400], F32, kind="ExternalOutput")

    RG = [list(range(N_CORES))]

    with tile.TileContext(nc) as tc:
        with (
            tc.tile_pool(name="w1p", bufs=15) as w1p,
            tc.tile_pool(name="w2p", bufs=6) as w2p,
            tc.tile_pool(name="const", bufs=1) as constp,
            tc.tile_pool(name="persist", bufs=1) as persist,
            tc.tile_pool(name="dram", bufs=1, space="DRAM") as dramp,
        ):
            # Constants.
            ident = constp.tile([64, 64], BF16)
            make_identity(nc, ident[:])
            ones1 = constp.tile([1, 64], BF16)
            nc.gpsimd.memset(ones1[:], 1.0)
            zero1 = constp.tile([1, 128], BF16)
            nc.gpsimd.memset(zero1[:], 0.0)
            ones512 = constp.tile([1, 512], BF16)
            nc.gpsimd.memset(ones512[:], 1.0)
            bg_sb = constp.tile([LATENT, 1], F32)
            dinv_d_sb = constp.tile([128, DPC], F32)
            b1_sb = constp.tile([128, M1], F32)
            b2_sb = constp.tile([1, N2], BF16)

            # Persistent SBUF tensors.
            zT = persist.tile([128, DPC], BF16)          # [latent, local node] *4
            gsb = persist.tile([16, GPC, CW], F8)        # packed G-tri, *16
            gT_big = persist.tile([128, K1, 64], F8)     # GT k-tiles x 64 graphs
            y1T = persist.tile([128, M1, 64], F8)        # y1^T m-tiles x graphs *8
            y2a = persist.tile([128, M2 * 32], BF16)     # z2^T partial *1024, half
            y2b = persist.tile([128, M2 * 32], BF16)

            # DRAM bounce buffers for the collectives.
            g_loc = dramp.tile([GPC, NTRI], F8, name="g_loc")
            g_all = dramp.tile([N_GRAPHS, NTRI], F8, addr_space="Shared",
                               name="g_all")
            y_loc = dramp.tile([128, M2 * 64], BF16, name="y_loc")
            y_red = dramp.tile([16, M2 * 64], BF16, name="y_red")

            # ---- Phase A: h = x @ Wg (fp8 DR), aggT = h'.T @ ats (fp8 DR) ----
            with (
                tc.tile_pool(name="xa", bufs=1) as xap,
                tc.tile_pool(name="h2p", bufs=8) as h2p,
                tc.tile_pool(name="hps", bufs=4, space="PSUM") as hps,
                tc.tile_pool(name="aggps", bufs=1, space="PSUM") as aggps,
            ):
                wg_sb = xap.tile([128, 2, LATENT], F8, tag="wg")
                nc.sync.dma_start(
                    wg_sb[:], wg[:, :].rearrange("(a b) c -> b a c", a=2))
                xt_sb = xap.tile([128, 2, N_NODES], F8, tag="xt")
                for half in range(2):
                    cs, ce = half * 2560, (half + 1) * 2560
                    xdma = nc.sync.dma_start(
                        xt_sb[:, :, cs:ce],
                        xT[:, cs:ce].rearrange("(a b) c -> b a c", a=2))
                    if half == 0:
                        first_xt = xdma.ins
                at_sb = xap.tile([128, 40, DPC], F8, tag="at")
                for t in range(5):
                    nc.sync.dma_start(
                        at_sb[:, 8 * t:8 * (t + 1), :],
                        ats[1024 * t:1024 * (t + 1), :].rearrange(
                            "(a b) c -> b a c", a=8))
                nc.sync.dma_start(dinv_d_sb[:], dinv_d[:, :])
                nc.sync.dma_start(bg_sb[:], bg4[:, :])
                nc.sync.dma_start(b1_sb[:], b1s[:, :])
                b2dma = nc.sync.dma_start(b2_sb[:], b2s[:, :])

                agg = aggps.tile([128, DPC], F32)
                for t in range(20):
                    ph = hps.tile([128, 2, 128], F32)
                    for kk in range(2):
                        k = 2 * t + kk
                        nc.tensor.matmul(ph[:, kk, :],
                                         lhsT=xt_sb[:, :, k * 128:(k + 1) * 128],
                                         rhs=wg_sb[:], start=True, stop=True,
                                         perf_mode=DR)
                    h2 = h2p.tile([128, 2, 128], F8)
                    if t % 2 == 0:
                        nc.vector.tensor_scalar_mul(h2[:], ph[:], S_H)
                    else:
                        nc.scalar.activation(h2[:], ph[:],
                                             mybir.ActivationFunctionType.Copy,
                                             scale=S_H)
                    st, sp = (t == 0), (t == 19)
                    nc.tensor.matmul(agg[:, 0:512], lhsT=h2[:],
                                     rhs=at_sb[:, 2 * t:2 * t + 2, 0:512],
                                     start=st, stop=sp, perf_mode=DR)
                    nc.tensor.matmul(agg[:, 512:640], lhsT=h2[:],
                                     rhs=at_sb[:, 2 * t:2 * t + 2, 512:640],
                                     start=st, stop=sp, perf_mode=DR)
                aggs = xap.tile([128, DPC], F32, tag="aggs")
                nc.vector.tensor_tensor(aggs[:], agg[:], dinv_d_sb[:],
                                        op=mybir.AluOpType.mult)
                nc.scalar.activation(zT[:], aggs[:],
                                     mybir.ActivationFunctionType.Relu,
                                     bias=bg_sb[:, 0:1])

            # ---- Phase B: block-triangle G per graph, pack, AllGather ----
            with tc.tile_pool(name="gps", bufs=4, space="PSUM") as gps:
                for g in range(GPC):
                    gtri = gps.tile([16, CW], F32)
                    base = g * N_NEURONS
                    for a in range(NB):
                        za = zT[:, base + a * BLK:base + (a + 1) * BLK]
                        rb = zT[:, base + a * BLK:base + N_NEURONS]
                        nc.tensor.matmul(gtri[:, C0[a]:C0[a] + (NB - a) * BLK],
                                         lhsT=za, rhs=rb, start=True, stop=True)
                    if g % 2 == 0:
                        nc.vector.tensor_copy(gsb[:, g, :], gtri[:])
                    else:
                        nc.scalar.activation(gsb[:, g, :], gtri[:],
                                             mybir.ActivationFunctionType.Copy)
                nc.scalar.dma_start(
                    g_loc[:, :].rearrange("g (i c) -> i g c", i=16), gsb[:])
                nc.gpsimd.collective_compute(
                    "AllGather", mybir.AluOpType.bypass, replica_groups=RG,
                    ins=[g_loc.opt()], outs=[g_all.opt()],
                )

            # ---- Phase C0/C1: transpose Gall into k-tiles; fc1 (DR) ----
            with (
                tc.tile_pool(name="gallp", bufs=3) as gallp,
                tc.tile_pool(name="tps", bufs=3, space="PSUM") as tps,
                tc.tile_pool(name="y1ps", bufs=1, space="PSUM") as y1psp,
            ):
                # fp8 transpose is rejected by walrus (needs elem step 2), so
                # convert each g_all tile to bf16 first; the PSUM->SBUF copy
                # casts back to fp8.
                ga_last = None
                for blk in range(3):
                    ga = gallp.tile([N_GRAPHS, 1280], F8, tag="ga8")
                    ga_last = nc.scalar.dma_start(
                        ga[:], g_all[:, blk * 1280:(blk + 1) * 1280])
                    gab = gallp.tile([N_GRAPHS, 1280], BF16, tag="gab")
                    if blk % 2 == 0:
                        nc.vector.tensor_copy(gab[:], ga[:])
                    else:
                        nc.scalar.activation(gab[:], ga[:],
                                             mybir.ActivationFunctionType.Copy)
                    for jp in range(5):
                        # bank-padded: HW matmul start=True zeroes its whole
                        # PSUM bank, so transpose tiles must not share banks
                        # with the fc1 accumulator.
                        tp = tps.tile([128, 16, 64], BF16)
                        for jj in range(2):
                            j = jp * 2 + jj
                            nc.tensor.transpose(tp[:, jj, :],
                                                gab[:, j * 128:(j + 1) * 128],
                                                ident[:])
                        t0 = blk * 10 + jp * 2
                        nc.vector.tensor_copy(gT_big[:, t0:t0 + 2, :],
                                              tp[:, 0:2, :])

                y1ps = y1psp.tile([128, 1024], F32)  # 2-bank padded
                # seed each PSUM bank exactly once (start=True zeroes the
                # whole bank on HW; interleaved starts wipe sibling regions)
                nc.tensor.matmul(y1ps[:, 0:512], lhsT=zero1[:],
                                 rhs=ones512[:], start=True, stop=False)
                nc.tensor.matmul(y1ps[:, 512:832], lhsT=zero1[:],
                                 rhs=ones512[:, 0:320], start=True, stop=False)
                for p in range(15):
                    w1t = w1p.tile([128, 2, HS], F8, tag="w1")
                    w1dma = nc.sync.dma_start(
                        w1t[:],
                        w1s[p * 256:(p + 1) * 256, :].rearrange(
                            "(a b) c -> b a c", a=2))
                    if p == 0:
                        add_dep_helper(w1dma.ins, first_xt, sync=True,
                                       reason="w-stream after first xT")
                    for m in range(M1):
                        mw = 128 if m < 12 else 64
                        nc.tensor.matmul(
                            y1ps[0:mw, m * 64:(m + 1) * 64],
                            lhsT=w1t[:, :, m * 128:m * 128 + mw],
                            rhs=gT_big[:, 2 * p:2 * p + 2, :],
                            start=False, stop=(p == 14), perf_mode=DR)
                for m in range(M1):
                    mw = 128 if m < 12 else 64
                    nc.scalar.activation(y1T[0:mw, m, :],
                                         y1ps[0:mw, m * 64:(m + 1) * 64],
                                         mybir.ActivationFunctionType.Relu,
                                         bias=b1_sb[0:mw, m:m + 1],
                                         scale=S_Y1 / (16.0 * S_W1))

            # ---- Phase C2: fc2 (DR), partials -> ReduceScatter -> sigmoid ----
            with (
                tc.tile_pool(name="z2ps", bufs=1, space="PSUM") as z2psp,
                tc.tile_pool(name="sig", bufs=1) as sigp,
            ):
                z2ps = z2psp.tile([128, M2 * 64], F32)
                for m in range(M2):
                    nc.tensor.matmul(z2ps[:, m * 64:(m + 1) * 64],
                                     lhsT=b2_sb[:, m * 128:(m + 1) * 128],
                                     rhs=ones1[:], start=True, stop=False)
                # W2 streamed as single-k-tile DMAs so the post-AllGather g_all
                # loads only wait one transfer for the DMA engines; k>=4 gated
                # behind those loads to keep FIFO order right.
                for p in range(KP2 + 1):
                    if p < KP2:
                        w2t = w2p.tile([128, 2, N2], F8, tag="w2")
                        for kk in range(2):
                            k = 2 * p + kk
                            w2dma = nc.sync.dma_start(
                                w2t[:, kk, :],
                                w2s[k * 128:(k + 1) * 128, :])
                            if k >= 4:
                                add_dep_helper(w2dma.ins, ga_last.ins, sync=True,
                                               reason="w2 tail after gall loads")
                        rhs = y1T[:, 2 * p:2 * p + 2, :]
                        for m in range(M2):
                            nc.tensor.matmul(z2ps[:, m * 64:(m + 1) * 64],
                                             lhsT=w2t[:, :, m * 128:(m + 1) * 128],
                                             rhs=rhs, start=False, stop=False,
                                             perf_mode=DR)
                    else:
                        w2t = w2p.tile([64, N2], F8, tag="w2last", bufs=1)
                        w2dma = nc.sync.dma_start(w2t[:], w2s[1536:1600, :])
                        add_dep_helper(w2dma.ins, ga_last.ins, sync=True,
                                       reason="w2 tail after gall loads")
                        rhs = y1T[0:64, 12, :]
                        for m in range(M2):
                            nc.tensor.matmul(z2ps[:, m * 64:(m + 1) * 64],
                                             lhsT=w2t[0:64, m * 128:(m + 1) * 128],
                                             rhs=rhs, start=False, stop=True)
                half = M2 * 64 // 2
                nc.vector.tensor_copy(y2a[:], z2ps[:, 0:half])
                nc.scalar.activation(y2b[:], z2ps[:, half:],
                                     mybir.ActivationFunctionType.Copy)
                nc.scalar.dma_start(y_loc[:, 0:half], y2a[:])
                nc.scalar.dma_start(y_loc[:, half:], y2b[:])
                nc.gpsimd.collective_compute(
                    "ReduceScatter", mybir.AluOpType.add, replica_groups=RG,
                    ins=[y_loc.opt()], outs=[y_red.opt()],
                )
                ys = sigp.tile([128, 400], BF16, tag="ys")
                nc.scalar.dma_start(
                    ys[:], y_red[:, :].rearrange("p (u v) -> (p u) v", u=8))
                yo = sigp.tile([128, 400], F32, tag="yo")
                nc.scalar.activation(yo[:], ys[:],
                                     mybir.ActivationFunctionType.Sigmoid,
                                     scale=1.0 / (S_Y1 * S_W2))
                nc.scalar.dma_start(y[:, :], yo[:])

    _fix_excess_waits(nc)
    return nc


_NC_CACHE = None


def _get_nc():
    global _NC_CACHE
    if _NC_CACHE is None:
        _NC_CACHE = build_nc()
    return _NC_CACHE


def _tri_index_maps():
    """vec index v = i*240 + c ; c -> (a, bb, j) via C0; returns row, col, sym."""
    v = np.arange(NTRI)
    i = v // CW
    c = v % CW
    c0 = np.asarray(C0)
    a = np.searchsorted(c0, c, side="right") - 1
    off = c - c0[a]
    bb = off // BLK
    j = off % BLK
    row = a * BLK + i
    col = (a + bb) * BLK + j
    return row, col, bb > 0


def prep_in_maps(x, edge_index, Wg, bg, W1, b1, W2, b2):
    x = np.asarray(x, np.float32)
    edge_index = np.asarray(edge_index)
    Wg = np.asarray(Wg, np.float32)
    bg = np.asarray(bg, np.float32)
    W1 = np.asarray(W1, np.float32)
    b1 = np.asarray(b1, np.float32)
    W2 = np.asarray(W2, np.float32)
    b2 = np.asarray(b2, np.float32)

    src = edge_index[0].astype(np.int64)
    dst = edge_index[1].astype(np.int64)

    deg = np.bincount(dst, minlength=N_NODES).astype(np.float32)
    dinv = np.where(deg > 0, 1.0 / np.sqrt(np.maximum(deg, 1.0)), 0.0).astype(np.float32)

    counts = np.bincount(src * N_NODES + dst, minlength=N_NODES * N_NODES)
    at = (counts.reshape(N_NODES, N_NODES).astype(np.float32)
          * (dinv * S_A)[:, None]).astype(NP_F8)

    xT_np = np.ascontiguousarray(x.T).astype(NP_F8)
    wg_np = Wg.astype(NP_F8)
    bg_np = np.ascontiguousarray((S_Z * bg).reshape(LATENT, 1))

    # W1' with symmetric fold, rows in device vec order, x128 fp8
    row, col, sym = _tri_index_maps()
    W1p = W1[row * N_NEURONS + col, :].copy()
    W1p[sym] += W1[col[sym] * N_NEURONS + row[sym], :]
    W1p = (W1p * S_W1).astype(NP_F8)

    w2_np = (W2 * S_W2).astype(NP_F8)
    b1_pad = np.zeros((128, M1), np.float32)
    b2_1024 = (b2 * S_Y1 * S_W2).astype(np.float32)

    in_maps = []
    for c in range(N_CORES):
        s0 = c * HS
        b1c = b1_pad.copy()
        bslice = b1[s0:s0 + HS] * S_Y1
        b1c[:, :12] = bslice[:1536].reshape(12, 128).T
        b1c[:64, 12] = bslice[1536:]
        b2c = b2_1024 if c == 0 else np.zeros_like(b2_1024)
        in_maps.append({
            "xT": xT_np,
            "wg": wg_np,
            "ats": np.ascontiguousarray(at[:, c * DPC:(c + 1) * DPC]),
            "dinv_d": np.ascontiguousarray(np.broadcast_to(
                (dinv * (S_Z / (S_H * S_A)))[c * DPC:(c + 1) * DPC], (128, DPC))),
            "bg4": bg_np,
            "w1s": np.ascontiguousarray(W1p[:, s0:s0 + HS]),
            "b1s": b1c,
            "w2s": np.ascontiguousarray(w2_np[s0:s0 + HS, :]),
            "b2s": np.ascontiguousarray(b2c.reshape(1, N2)).astype(NP_BF16),
        })
    return in_maps


def kernel(x, edge_index, Wg, bg, W1, b1, W2, b2):
    in_maps = prep_in_maps(x, edge_index, Wg, bg, W1, b1, W2, b2)
    nc = _get_nc()
    res = run_bass_kernel_spmd(nc, in_maps, core_ids=list(range(N_CORES)))
    # y_c [128, 400] f32: rows q = p'*8+u (p' in 0..15), cols v;
    # value = sigmoid(z2[graph g, n2 = m*128 + 16c + p']) with
    # m = (u*400+v)//64, g = (u*400+v)%64.
    out = np.empty((N_GRAPHS, N2), np.float32)
    p_idx = np.arange(16)
    m_idx = np.arange(M2)
    for c in range(N_CORES):
        yc = np.asarray(res.results[c]["y"], np.float32)
        f = yc.reshape(16, 8 * 400).reshape(16, M2, 64)   # [p', m, g]
        n2idx = m_idx[None, :] * 128 + 16 * c + p_idx[:, None]   # [16, 50]
        out[:, n2idx] = f.transpose(2, 0, 1)
    return out.reshape(-1)
